# revision 23
# baseline (speedup 1.0000x reference)
"""Trainium2 Bass kernel for BPUMultiHeadedAttention (16 heads, dk=64,
chunk=512, time=8192) sharded over heads across 8 NeuronCores (2 heads/core).

Per-core device program (all layouts chosen so no on-device transposes are
needed):
  - q/k projections in (feat, time) orientation; v projection emitted
    transposed as (time, feat) by swapping matmul operands.
  - v-cache is transposed on the host into (time, feat) with a ones column
    appended per head, so the attention*V matmul (M=65) accumulates both
    x = v @ exp(scores) and sumexp = 1 @ exp(scores) in one PSUM tile.
  - scores are computed per 128-wide time tile in (t, c) orientation with
    the two heads row-tiled on the PE array (K=64 each), drained through
    one ScalarE Exp activation spanning both PSUM banks.
  - softmax normalization is deferred: x is divided by sumexp after the
    full accumulation (a K=1 broadcast matmul + one elementwise multiply).
  - a single AllGather of the per-core x (128, 512) gives every core the
    full (1024, 512) attention output; each core then computes its own 128
    rows of the final linear_out (column-parallel), so the host only
    concatenates.
"""

import os
import math
import numpy as np

H, DK, CHUNK, LEFT = 16, 64, 512, 15
NFEAT = H * DK            # 1024
CACHE_T = CHUNK * LEFT    # 7680
TIME = CHUNK * (LEFT + 1) # 8192
DENOM = 1.0 / math.sqrt(DK)

NCORES = 8
HPC = H // NCORES         # heads per core = 2
FPC = HPC * DK            # features per core = 128
NT = TIME // 128          # 64 time tiles
NT_CACHE = CACHE_T // 128 # 60 cached time tiles
KCH = NFEAT // 128        # 8 contraction chunks

# Matmul operand dtype: bf16 (1 cyc/row on the PE + fast weight load) by
# default; f32r ("KERNEL_BF16=0") runs as two-pass fp32-HIGH at half rate.
USE_BF16 = os.environ.get("KERNEL_BF16", "1") != "0"
USE_F32R = os.environ.get("KERNEL_F32R", "1") != "0"
# Device AllGather + column-parallel linear_out vs row-parallel linear_out
# with the 8 partial sums reduced on the host (the AllGather measures ~40us
# on this fabric, an order above its documented cost, so host-reduce wins).
DEVICE_AG = os.environ.get("KERNEL_DEVICE_AG", "0") != "0"

_NC = None
LAST = None  # BassKernelResults of the most recent device run (for test.py)


def _build():
    import concourse.bass as bass
    import concourse.bacc as bacc
    import concourse.mybir as mybir
    import concourse.tile as tile
    from concourse.bass import ts

    f32 = mybir.dt.float32
    fr = mybir.dt.bfloat16 if USE_BF16 else (mybir.dt.float32r if USE_F32R else f32)

    nc = bacc.Bacc("TRN2", target_bir_lowering=False, num_devices=NCORES)

    # ---- DRAM I/O (per-core shapes) ----
    xq_d = nc.dram_tensor("xq", [KCH, 128, CHUNK], fr, kind="ExternalInput")
    xk_d = nc.dram_tensor("xk", [KCH, 128, CHUNK], fr, kind="ExternalInput")
    xv_d = nc.dram_tensor("xv", [KCH, 128, CHUNK], fr, kind="ExternalInput")
    wq_d = nc.dram_tensor("wq", [KCH, 128, FPC], fr, kind="ExternalInput")
    wk_d = nc.dram_tensor("wk", [KCH, 128, FPC], fr, kind="ExternalInput")
    wv_d = nc.dram_tensor("wv", [KCH, 128, FPC], fr, kind="ExternalInput")
    wo_d = nc.dram_tensor("wo", [KCH, 128, FPC], fr, kind="ExternalInput")
    cst32_d = nc.dram_tensor("cst32", [128, 3], f32, kind="ExternalInput")
    cstb_d = nc.dram_tensor("cstb", [1, 384], fr, kind="ExternalInput")
    onec_d = nc.dram_tensor("onec_r", [128, 1], fr, kind="ExternalInput")
    kc_d = nc.dram_tensor("kcache", [128, CACHE_T], fr, kind="ExternalInput")
    vt_d = nc.dram_tensor("vtcache", [CACHE_T, 130], fr, kind="ExternalInput")

    if DEVICE_AG:
        out_d = nc.dram_tensor("out_block", [128, CHUNK], f32, kind="ExternalOutput")
        # collective bounce buffers
        ccin_d = nc.dram_tensor("ccin", [128, CHUNK], fr)
        ccout_d = nc.dram_tensor("ccout", [NFEAT, CHUNK], fr, addr_space="Shared")
    else:
        out_d = nc.dram_tensor("out_block", [KCH, 128, CHUNK], fr, kind="ExternalOutput")
    knew_d = nc.dram_tensor("knew", [128, CHUNK], f32, kind="ExternalOutput")
    vnt_d = nc.dram_tensor("vnt", [4, 128, 128], f32, kind="ExternalOutput")

    Exp = mybir.ActivationFunctionType.Exp
    Ident = mybir.ActivationFunctionType.Identity


    with tile.TileContext(nc) as tc:
        with (
            tc.tile_pool(name="cst", bufs=1) as cst,
            tc.tile_pool(name="big", bufs=1) as big,
        ):
            # ---- persistent SBUF tiles ----
            xq_sb = big.tile([128, KCH, CHUNK], fr, tag="xq")
            xk_sb = big.tile([128, KCH, CHUNK], fr, tag="xk")
            xv_sb = big.tile([128, KCH, CHUNK], fr, tag="xv")
            wq_sb = big.tile([128, KCH, FPC], fr, tag="wq")
            wk_sb = big.tile([128, KCH, FPC], fr, tag="wk")
            wv_sb = big.tile([128, KCH, FPC], fr, tag="wv")
            wo_sb = big.tile([128, KCH, FPC], fr, tag="wo")
            k_sb = big.tile([128, TIME], fr, tag="ksb")
            vt_sb = big.tile([128, NT, 130], fr, tag="vtsb")
            q_sb = big.tile([128, CHUNK], fr, tag="qsb")
            if DEVICE_AG:
                xf_sb = big.tile([128, KCH, CHUNK], fr, tag="xf")

            cst32_sb = cst.tile([128, 3], f32, tag="cst32")
            cstb_sb = cst.tile([1, 384], fr, tag="cstb")
            onec_sb = cst.tile([128, 1], fr, tag="onec")
            bqd_sb = cst32_sb[:, 0:1]
            bk_sb = cst32_sb[:, 1:2]
            bo_sb = cst32_sb[:, 2:3]
            bv_sb = cstb_sb[0:1, 0:128]
            ones1 = cstb_sb[0:1, 128:256]
            sel_t = cst.tile([65, 128], fr, tag="sel")
            xn1 = cst.tile([64, CHUNK], fr, tag="xn1")
            xn_full = cst.tile([128, CHUNK], fr, tag="xnf")
            bp_sb = cst.tile([128, 2, CHUNK], f32, tag="bpsb")
            o_sb = cst.tile([128, CHUNK], f32, tag="osb")
            kout_sb = cst.tile([128, CHUNK], f32, tag="kout")
            vout_sb = cst.tile([128, 4, 128], f32, tag="vout")

            # ---- input DMAs: few big transfers; cache streams on the
            # gpsimd queue so it doesn't serialize behind the x/w loads ----
            xq_r = xq_d[:].rearrange("k p n -> p k n")
            xk_r = xk_d[:].rearrange("k p n -> p k n")
            xv_r = xv_d[:].rearrange("k p n -> p k n")
            for i in range(4):
                nc.sync.dma_start(out=xq_sb[:, 2*i:2*i+2, :], in_=xq_r[:, 2*i:2*i+2, :])
            nc.sync.dma_start(out=wq_sb[:, :, :], in_=wq_d[:].rearrange("k p d -> p k d"))
            nc.sync.dma_start(out=cst32_sb[:, :], in_=cst32_d[:, :])
            for i in range(4):
                nc.sync.dma_start(out=xk_sb[:, 2*i:2*i+2, :], in_=xk_r[:, 2*i:2*i+2, :])
            nc.sync.dma_start(out=wk_sb[:, :, :], in_=wk_d[:].rearrange("k p d -> p k d"))
            nc.sync.dma_start(out=cstb_sb[:, :], in_=cstb_d[:, :])
            nc.sync.dma_start(out=onec_sb[:, :], in_=onec_d[:, :])
            for i in range(4):
                nc.sync.dma_start(out=xv_sb[:, 2*i:2*i+2, :], in_=xv_r[:, 2*i:2*i+2, :])
            nc.sync.dma_start(out=wv_sb[:, :, :], in_=wv_d[:].rearrange("k p d -> p k d"))
            nc.sync.dma_start(out=wo_sb[:, :, :], in_=wo_d[:].rearrange("k p d -> p k d"))
            # k cache and vT cache in 6 chunks each on the hardware DGE
            # (software DGE via gpsimd measured ~3 GB/s per engine)
            KCHUNK = 1280
            vt_r = vt_d[:].rearrange("(j p) d -> p j d", p=128)
            for i in range(CACHE_T // KCHUNK):
                nc.sync.dma_start(
                    out=k_sb[:, i * KCHUNK:(i + 1) * KCHUNK],
                    in_=kc_d[:, i * KCHUNK:(i + 1) * KCHUNK],
                )
                nc.sync.dma_start(
                    out=vt_sb[:, i * 10:(i + 1) * 10, :],
                    in_=vt_r[:, i * 10:(i + 1) * 10, :],
                )
            # sel row (ones in first 64 cols) placed at partition 64 for the
            # broadcast matmul's lhsT
            nc.sync.dma_start(out=sel_t[64:65, :], in_=cstb_d[0:1, 256:384])

            # ---- attention (q/k/v projections interleaved into the
            # score-group stream so the PE never idles on their DMAs) ----
            with (
                tc.tile_pool(name="scp", bufs=2, space="PSUM") as scp,
                tc.tile_pool(name="xap", bufs=1, space="PSUM") as xap,
                tc.tile_pool(name="exp", bufs=6) as exp_pool,
            ):
                x0 = xap.tile([65, CHUNK], f32, tag="x0")
                x1 = xap.tile([65, CHUNK], f32, tag="x1")

                # q projection (uses one score slot, then releases it)
                q_ps = scp.tile([128, 3, CHUNK], f32, tag="sc")
                for kc in range(KCH):
                    nc.tensor.matmul(
                        q_ps[:, 0, :], wq_sb[:, kc, :], xq_sb[:, kc, :],
                        start=(kc == 0), stop=(kc == KCH - 1),
                    )
                # q = (raw + bq) * denom, with bqd = bq*denom pre-folded
                nc.scalar.activation(q_sb[:, :], q_ps[:, 0, :], Ident,
                                     bias=bqd_sb[:, 0:1], scale=DENOM)

                def k_proj():
                    k_ps = scp.tile([128, 3, CHUNK], f32, tag="sc")
                    for kc in range(KCH):
                        nc.tensor.matmul(
                            k_ps[:, 0, :], wk_sb[:, kc, :], xk_sb[:, kc, :],
                            start=(kc == 0), stop=(kc == KCH - 1),
                        )
                    nc.scalar.activation(kout_sb[:, :], k_ps[:, 0, :], Ident,
                                         bias=bk_sb[:, 0:1], scale=1.0)
                    nc.vector.tensor_copy(k_sb[:, CACHE_T:TIME], kout_sb[:, :])
                    nc.sync.dma_start(out=knew_d[:, :], in_=kout_sb[:, :])

                def v_proj(j):
                    # transposed: vT[t, d] = sum_f x[f, t] WvT[f, d] + bv[d]
                    v_ps = scp.tile([128, 3, CHUNK], f32, tag="sc")
                    vp = v_ps[:, 0, 0:128]
                    for kc in range(KCH):
                        nc.tensor.matmul(
                            vp, xv_sb[:, kc, j * 128:(j + 1) * 128],
                            wv_sb[:, kc, :],
                            start=(kc == 0), stop=False,
                        )
                    nc.tensor.matmul(vp, ones1[:, :], bv_sb[:, :],
                                     start=False, stop=True)
                    jj = NT_CACHE + j
                    nc.vector.tensor_copy(vt_sb[:, jj, 64:65], onec_sb[:, :])
                    nc.vector.tensor_copy(vt_sb[:, jj, 129:130], onec_sb[:, :])
                    nc.vector.tensor_copy(vt_sb[:, jj, 0:64], vp[:, 0:64])
                    nc.vector.tensor_copy(vt_sb[:, jj, 65:129], vp[:, 64:128])
                    nc.vector.tensor_copy(vout_sb[:, j, :], vp)
                # scores for 3 t-tiles = 6 psum banks = 2 pool slots of 3
                # banks; one Exp ACTIVATE drains 3 banks (1536 elems/lane),
                # amortizing the ~350-cycle ACT fixed cost over 1.5 tiles.
                # slot A holds (h0,j0),(h1,j0),(h0,j1); B holds
                # (h1,j1),(h0,j2),(h1,j2).
                ex_slots = {}

                def qk_pair(j, d0, d1):
                    # one t-tile's two heads, row-tiled concurrently on the
                    # PE; destinations may live in different psum slots
                    nc.tensor.matmul(
                        d0, k_sb[0:64, ts(j, 128)], q_sb[0:64, :],
                        start=True, stop=True)
                    nc.tensor.matmul(
                        d1, k_sb[64:128, ts(j, 128)], q_sb[64:128, :],
                        start=True, stop=True)

                def av(j):
                    exa, exb = ex_slots.pop(j)
                    g = j % 3
                    if g == 0:
                        r0, r1 = exa[:, 0, :], exa[:, 1, :]
                    elif g == 1:
                        r0, r1 = exa[:, 2, :], exb[:, 0, :]
                    else:
                        r0, r1 = exb[:, 1, :], exb[:, 2, :]
                    nc.tensor.matmul(
                        x0[:, :], vt_sb[:, j, 0:65], r0,
                        start=(j == 0), stop=(j == NT - 1))
                    nc.tensor.matmul(
                        x1[:, :], vt_sb[:, j, 65:130], r1,
                        start=(j == 0), stop=(j == NT - 1))

                LAG = 3  # in t-tiles, rounded up to whole 3-tile groups
                NG = NT // 3  # 21 full groups; tile 63 handled separately
                done = 0
                for g in range(NG):
                    j = 3 * g
                    scA = scp.tile([128, 3, CHUNK], f32, tag="sc")
                    scB = scp.tile([128, 3, CHUNK], f32, tag="sc")
                    qk_pair(j, scA[:, 0, :], scA[:, 1, :])
                    qk_pair(j + 1, scA[:, 2, :], scB[:, 0, :])
                    qk_pair(j + 2, scB[:, 1, :], scB[:, 2, :])
                    exa = exp_pool.tile([128, 3, CHUNK], fr, tag="ex")
                    nc.scalar.activation(exa[:, :, :], scA[:, :, :], Exp)
                    exb = exp_pool.tile([128, 3, CHUNK], fr, tag="ex")
                    nc.scalar.activation(exb[:, :, :], scB[:, :, :], Exp)
                    ex_slots[j] = (exa, exb)
                    ex_slots[j + 1] = (exa, exb)
                    ex_slots[j + 2] = (exa, exb)
                    while done <= 3 * g + 2 - LAG:
                        av(done)
                        done += 1
                    if g == 1:
                        k_proj()
                    elif 3 <= g <= 6:
                        v_proj(g - 3)
                    elif g == 7:
                        nc.sync.dma_start(
                            out=vnt_d[:].rearrange("j p d -> p j d"),
                            in_=vout_sb[:, :, :],
                        )
                # last tile (63): 2 banks in a smaller slot
                scL = scp.tile([128, 3, CHUNK], f32, tag="sc")
                qk_pair(NT - 1, scL[:, 0, :], scL[:, 1, :])
                exl = exp_pool.tile([128, 3, CHUNK], fr, tag="ex")
                nc.scalar.activation(exl[:, 0:2, :], scL[:, 0:2, :], Exp)
                ex_slots[NT - 1] = (exl, exl)
                while done < NT - 1:
                    av(done)
                    done += 1
                # final tile uses exl directly
                nc.tensor.matmul(x0[:, :], vt_sb[:, NT - 1, 0:65], exl[:, 0, :],
                                 start=False, stop=True)
                nc.tensor.matmul(x1[:, :], vt_sb[:, NT - 1, 65:130], exl[:, 1, :],
                                 start=False, stop=True)

                # ---- softmax normalization (deferred) ----
                # The sumexp rows live on one partition; a 1-lane DVE
                # reciprocal costs ~4us, so bounce them through a (128, 8)
                # layout via SBUF->SBUF DMA to use all 128 lanes.
                sr_sb = cst.tile([65, 2 * CHUNK], f32, tag="srsb")
                nc.vector.tensor_copy(sr_sb[64:65, 0:CHUNK], x0[64:65, :])
                nc.vector.tensor_copy(sr_sb[64:65, CHUNK:2 * CHUNK], x1[64:65, :])
                rr_sb = cst.tile([128, 8], f32, tag="rrsb")
                nc.sync.dma_start(out=rr_sb[:, :], in_=sr_sb[64:65, :])
                rq_sb = cst.tile([128, 8], fr, tag="rqsb")
                with nc.allow_low_precision(reason="softmax 1/sumexp in bf16"):
                    nc.vector.reciprocal(rq_sb[:, :], rr_sb[:, :])
                rs_c = cst.tile([65, 2 * CHUNK], fr, tag="rsc")
                nc.sync.dma_start(out=rs_c[64:65, :], in_=rq_sb[:, :])
                rs_t0 = rs_c[64:65, 0:CHUNK]
                rs_t1 = rs_c[64:65, CHUNK:2 * CHUNK]
                bp = scp.tile([128, 2, CHUNK], f32, tag="sc")
                nc.tensor.matmul(bp[:, 0, :], sel_t[64:65, :], rs_t0,
                                 start=True, stop=True)
                nc.tensor.matmul(bp[:, 1, :], sel_t[64:65, :], rs_t1,
                                 start=True, stop=True)
                nc.vector.tensor_copy(bp_sb[:, :, :], bp[:, :, :])
                nc.vector.tensor_mul(xn_full[0:64, :], x0[0:64, :], bp_sb[0:64, 0, :])
                nc.vector.tensor_mul(xn1[:, :], x1[0:64, :], bp_sb[0:64, 1, :])

            if DEVICE_AG:
                # all-gather x across cores, then column-parallel linear_out
                nc.sync.dma_start(out=ccin_d[0:64, :], in_=xn_full[0:64, :])
                nc.sync.dma_start(out=ccin_d[64:128, :], in_=xn1[:, :])
                nc.gpsimd.collective_compute(
                    "AllGather",
                    mybir.AluOpType.bypass,
                    replica_groups=[list(range(NCORES))],
                    ins=[ccin_d.ap().opt()],
                    outs=[ccout_d.ap().opt()],
                )
                nc.sync.dma_start(
                    out=xf_sb[:, :, :],
                    in_=ccout_d[:].rearrange("(k p) n -> p k n", p=128))
                with tc.tile_pool(name="pop", bufs=1, space="PSUM") as pop:
                    o_ps = pop.tile([128, CHUNK], f32, tag="o_ps")
                    for kc in range(KCH):
                        nc.tensor.matmul(
                            o_ps[:, :], wo_sb[:, kc, :], xf_sb[:, kc, :],
                            start=(kc == 0), stop=(kc == KCH - 1),
                        )
                    nc.scalar.activation(o_sb[:, :], o_ps[:, :], Ident,
                                         bias=bo_sb[:, 0:1], scale=1.0)
                nc.sync.dma_start(out=out_d[:, :], in_=o_sb[:, :])
            else:
                # row-parallel linear_out: this core's 128 features produce a
                # full (1024, 512) partial; the host sums the 8 partials.
                nc.sync.dma_start(out=xn_full[64:128, :], in_=xn1[:, :])
                o8_sb = cst.tile([128, KCH, CHUNK], fr, tag="o8")
                with tc.tile_pool(name="pop", bufs=2, space="PSUM") as pop:
                    for mb in range(KCH):
                        o_ps = pop.tile([128, CHUNK], f32, tag="o_ps")
                        nc.tensor.matmul(o_ps[:, :], wo_sb[:, mb, :], xn_full[:, :],
                                         start=True, stop=True)
                        if mb % 2 == 0:
                            nc.vector.tensor_copy(o8_sb[:, mb, :], o_ps[:, :])
                        else:
                            nc.scalar.copy(o8_sb[:, mb, :], o_ps[:, :])
                for i in range(4):
                    nc.sync.dma_start(
                        out=out_d[:].rearrange("k p n -> p k n")[:, 2*i:2*i+2, :],
                        in_=o8_sb[:, 2*i:2*i+2, :])

    nc.finalize()
    return nc


def _get_nc():
    global _NC
    if _NC is None:
        _NC = _build()
    return _NC


def _mm_np_dtype():
    if USE_BF16:
        import ml_dtypes
        return ml_dtypes.bfloat16
    return np.float32


def _prep_in_maps(q, k, v, cache, Wq, bq, Wk, bk, Wv, bv, Wo, bo):
    md = _mm_np_dtype()
    xq = np.ascontiguousarray(q.reshape(NFEAT, CHUNK)).astype(md).reshape(KCH, 128, CHUNK)
    xk = np.ascontiguousarray(k.reshape(NFEAT, CHUNK)).astype(md).reshape(KCH, 128, CHUNK)
    xv = np.ascontiguousarray(v.reshape(NFEAT, CHUNK)).astype(md).reshape(KCH, 128, CHUNK)
    in_maps = []
    for c in range(NCORES):
        rows = slice(128 * c, 128 * (c + 1))
        wq_c = np.ascontiguousarray(Wq[rows, :].T).astype(md).reshape(KCH, 128, FPC)
        wk_c = np.ascontiguousarray(Wk[rows, :].T).astype(md).reshape(KCH, 128, FPC)
        wv_c = np.ascontiguousarray(Wv[rows, :].T).astype(md).reshape(KCH, 128, FPC)
        if DEVICE_AG:
            wo_c = np.ascontiguousarray(Wo[rows, :].T).astype(md).reshape(KCH, 128, FPC)
        else:
            wo_c = np.ascontiguousarray(
                Wo[:, rows].T.reshape(128, KCH, 128).transpose(1, 0, 2)).astype(md)
        kc_c = np.ascontiguousarray(
            cache[0, 2 * c:2 * c + 2, 0:DK, :]).reshape(128, CACHE_T).astype(md)
        vt_c = np.empty((CACHE_T, 130), md)
        vt_c[:, 64] = 1.0
        vt_c[:, 129] = 1.0
        vt_c[:, 0:64] = cache[0, 2 * c, DK:2 * DK, :].T.astype(md)
        vt_c[:, 65:129] = cache[0, 2 * c + 1, DK:2 * DK, :].T.astype(md)
        cst32 = np.stack([bq[rows] * DENOM, bk[rows], bo[rows]], axis=1).astype(np.float32)
        cstb = np.zeros((1, 384), md)
        cstb[0, 0:128] = bv[rows].astype(md)
        cstb[0, 128:256] = 1.0
        cstb[0, 256:320] = 1.0
        in_maps.append({
            "xq": xq, "xk": xk, "xv": xv,
            "wq": wq_c, "wk": wk_c, "wv": wv_c, "wo": wo_c,
            "cst32": cst32,
            "cstb": cstb,
            "onec_r": np.ones((128, 1), md),
            "kcache": kc_c,
            "vtcache": vt_c,
        })
    return in_maps


def _assemble(results, cache, bo=None):
    x_out = np.empty((1, NFEAT, 1, CHUNK), np.float32)
    new_cache = np.empty((1, H, 2 * DK, TIME), np.float32)
    new_cache[0, :, 0:DK, 0:CACHE_T] = cache[0, :, 0:DK, :]
    new_cache[0, :, DK:2 * DK, 0:CACHE_T] = cache[0, :, DK:2 * DK, :]
    if not DEVICE_AG:
        acc = results[0]["out_block"].astype(np.float32).reshape(NFEAT, CHUNK).copy()
        for c in range(1, NCORES):
            acc += results[c]["out_block"].reshape(NFEAT, CHUNK)
        acc += bo[:, None]
        x_out[0, :, 0, :] = acc
    for c in range(NCORES):
        res = results[c]
        if DEVICE_AG:
            x_out[0, 128 * c:128 * (c + 1), 0, :] = res["out_block"]
        knew = res["knew"].reshape(HPC, DK, CHUNK)
        new_cache[0, 2 * c:2 * c + 2, 0:DK, CACHE_T:] = knew
        vnt = res["vnt"].reshape(4 * 128, 128)  # (t_new, 2*dk)
        new_cache[0, 2 * c, DK:2 * DK, CACHE_T:] = vnt[:, 0:64].T
        new_cache[0, 2 * c + 1, DK:2 * DK, CACHE_T:] = vnt[:, 64:128].T
    return x_out, new_cache


def _fallback(q, k, v, mask, cache, Wq, bq, Wk, bk, Wv, bv, Wo, bo):
    """Pure-numpy reference path (only used if mask is not all-ones)."""
    def lin(x, W, b):
        return np.einsum("oc,bcut->bout", W, x) + b[None, :, None, None]

    qp = lin(q, Wq, bq).reshape(1, H, DK, CHUNK)
    kp = lin(k, Wk, bk).reshape(1, H, DK, CHUNK)
    vp = lin(v, Wv, bv).reshape(1, H, DK, CHUNK)
    qp = np.transpose(qp, (0, 1, 3, 2))
    k_cache, v_cache = cache[:, :, 0:DK, :], cache[:, :, DK:, :]
    kk = np.concatenate([k_cache, kp], axis=3)
    vv = np.concatenate([v_cache, vp], axis=3)
    new_cache = np.concatenate([kk, vv], axis=2)
    scores = np.einsum("bhcd,bhdt->bhct", qp, kk) * DENOM
    neg = ~mask
    scores = np.where(neg, -np.inf, scores)
    m = scores.max(axis=-1, keepdims=True)
    e = np.exp(scores - m)
    attn = e / e.sum(axis=-1, keepdims=True)
    attn = np.where(neg, 0.0, attn)
    x = np.einsum("bhdt,bhct->bhdc", vv, attn)
    x = x.reshape(1, NFEAT, 1, CHUNK)
    x_out = lin(x, Wo, bo)
    return x_out.astype(np.float32), new_cache.astype(np.float32)


def kernel(q, k, v, mask, cache, Wq, bq, Wk, bk, Wv, bv, Wo, bo):
    global LAST
    q = np.asarray(q, np.float32)
    k = np.asarray(k, np.float32)
    v = np.asarray(v, np.float32)
    cache = np.asarray(cache, np.float32)
    Wq = np.asarray(Wq, np.float32); bq = np.asarray(bq, np.float32)
    Wk = np.asarray(Wk, np.float32); bk = np.asarray(bk, np.float32)
    Wv = np.asarray(Wv, np.float32); bv = np.asarray(bv, np.float32)
    Wo = np.asarray(Wo, np.float32); bo = np.asarray(bo, np.float32)
    mask_np = np.asarray(mask)
    if not mask_np.all():
        return _fallback(q, k, v, mask_np, cache, Wq, bq, Wk, bk, Wv, bv, Wo, bo)

    from concourse.bass_utils import run_bass_kernel_spmd

    nc = _get_nc()
    in_maps = _prep_in_maps(q, k, v, cache, Wq, bq, Wk, bk, Wv, bv, Wo, bo)
    trace = os.environ.get("KERNEL_TRACE", "0") != "0"
    LAST = run_bass_kernel_spmd(nc, in_maps, list(range(NCORES)), trace=trace)
    return _assemble(LAST.results, cache, bo)


# revision 24
# speedup vs baseline: 1.0879x; 1.0879x over previous
"""Trainium2 Bass kernel for BPUMultiHeadedAttention (16 heads, dk=64,
chunk=512, time=8192) sharded over heads across 8 NeuronCores (2 heads/core).

Per-core device program (all layouts chosen so no on-device transposes are
needed):
  - q/k projections in (feat, time) orientation; v projection emitted
    transposed as (time, feat) by swapping matmul operands.
  - v-cache is transposed on the host into (time, feat) with a ones column
    appended per head, so the attention*V matmul (M=65) accumulates both
    x = v @ exp(scores) and sumexp = 1 @ exp(scores) in one PSUM tile.
  - scores are computed per 128-wide time tile in (t, c) orientation with
    the two heads row-tiled on the PE array (K=64 each), drained through
    one ScalarE Exp activation spanning both PSUM banks.
  - softmax normalization is deferred: x is divided by sumexp after the
    full accumulation (a K=1 broadcast matmul + one elementwise multiply).
  - a single AllGather of the per-core x (128, 512) gives every core the
    full (1024, 512) attention output; each core then computes its own 128
    rows of the final linear_out (column-parallel), so the host only
    concatenates.
"""

import os
import math
import numpy as np

H, DK, CHUNK, LEFT = 16, 64, 512, 15
NFEAT = H * DK            # 1024
CACHE_T = CHUNK * LEFT    # 7680
TIME = CHUNK * (LEFT + 1) # 8192
DENOM = 1.0 / math.sqrt(DK)

NCORES = 8
HPC = H // NCORES         # heads per core = 2
FPC = HPC * DK            # features per core = 128
NT = TIME // 128          # 64 time tiles
NT_CACHE = CACHE_T // 128 # 60 cached time tiles
KCH = NFEAT // 128        # 8 contraction chunks

# Matmul operand dtype: bf16 (1 cyc/row on the PE + fast weight load) by
# default; f32r ("KERNEL_BF16=0") runs as two-pass fp32-HIGH at half rate.
USE_BF16 = os.environ.get("KERNEL_BF16", "1") != "0"
USE_F32R = os.environ.get("KERNEL_F32R", "1") != "0"
# Device AllGather + column-parallel linear_out vs row-parallel linear_out
# with the 8 partial sums reduced on the host (the AllGather measures ~40us
# on this fabric, an order above its documented cost, so host-reduce wins).
DEVICE_AG = os.environ.get("KERNEL_DEVICE_AG", "0") != "0"

_NC = None
LAST = None  # BassKernelResults of the most recent device run (for test.py)


def _build():
    import concourse.bass as bass
    import concourse.bacc as bacc
    import concourse.mybir as mybir
    import concourse.tile as tile
    from concourse.bass import ts

    f32 = mybir.dt.float32
    fr = mybir.dt.bfloat16 if USE_BF16 else (mybir.dt.float32r if USE_F32R else f32)

    nc = bacc.Bacc("TRN2", target_bir_lowering=False, num_devices=NCORES)

    # ---- DRAM I/O (per-core shapes) ----
    xq_d = nc.dram_tensor("xq", [KCH, 128, CHUNK], fr, kind="ExternalInput")
    xk_d = nc.dram_tensor("xk", [KCH, 128, CHUNK], fr, kind="ExternalInput")
    xv_d = nc.dram_tensor("xv", [KCH, 128, CHUNK], fr, kind="ExternalInput")
    wq_d = nc.dram_tensor("wq", [KCH, 128, FPC], fr, kind="ExternalInput")
    wk_d = nc.dram_tensor("wk", [KCH, 128, FPC], fr, kind="ExternalInput")
    wv_d = nc.dram_tensor("wv", [KCH, 128, FPC], fr, kind="ExternalInput")
    wo_d = nc.dram_tensor("wo", [KCH, 128, FPC], fr, kind="ExternalInput")
    cst32_d = nc.dram_tensor("cst32", [128, 3], f32, kind="ExternalInput")
    cstb_d = nc.dram_tensor("cstb", [1, 384], fr, kind="ExternalInput")
    onec_d = nc.dram_tensor("onec_r", [128, 1], fr, kind="ExternalInput")
    kc_d = nc.dram_tensor("kcache", [128, CACHE_T], fr, kind="ExternalInput")
    vt_d = nc.dram_tensor("vtcache", [CACHE_T, 130], fr, kind="ExternalInput")

    if DEVICE_AG:
        out_d = nc.dram_tensor("out_block", [128, CHUNK], f32, kind="ExternalOutput")
        # collective bounce buffers
        ccin_d = nc.dram_tensor("ccin", [128, CHUNK], fr)
        ccout_d = nc.dram_tensor("ccout", [NFEAT, CHUNK], fr, addr_space="Shared")
    else:
        out_d = nc.dram_tensor("out_block", [KCH, 128, CHUNK], fr, kind="ExternalOutput")
    knew_d = nc.dram_tensor("knew", [128, CHUNK], f32, kind="ExternalOutput")
    vnt_d = nc.dram_tensor("vnt", [4, 128, 128], f32, kind="ExternalOutput")

    Exp = mybir.ActivationFunctionType.Exp
    Ident = mybir.ActivationFunctionType.Identity


    with tile.TileContext(nc) as tc:
        with (
            tc.tile_pool(name="cst", bufs=1) as cst,
            tc.tile_pool(name="big", bufs=1) as big,
        ):
            # ---- persistent SBUF tiles ----
            xq_sb = big.tile([128, KCH, CHUNK], fr, tag="xq")
            xk_sb = big.tile([128, KCH, CHUNK], fr, tag="xk")
            xv_sb = big.tile([128, KCH, CHUNK], fr, tag="xv")
            wq_sb = big.tile([128, KCH, FPC], fr, tag="wq")
            wk_sb = big.tile([128, KCH, FPC], fr, tag="wk")
            wv_sb = big.tile([128, KCH, FPC], fr, tag="wv")
            wo_sb = big.tile([128, KCH, FPC], fr, tag="wo")
            k_sb = big.tile([128, TIME], fr, tag="ksb")
            vt_sb = big.tile([128, NT, 130], fr, tag="vtsb")
            q_sb = big.tile([128, CHUNK], fr, tag="qsb")
            if DEVICE_AG:
                xf_sb = big.tile([128, KCH, CHUNK], fr, tag="xf")

            cst32_sb = cst.tile([128, 3], f32, tag="cst32")
            cstb_sb = cst.tile([1, 384], fr, tag="cstb")
            onec_sb = cst.tile([128, 1], fr, tag="onec")
            bqd_sb = cst32_sb[:, 0:1]
            bk_sb = cst32_sb[:, 1:2]
            bo_sb = cst32_sb[:, 2:3]
            bv_sb = cstb_sb[0:1, 0:128]
            ones1 = cstb_sb[0:1, 128:256]
            sel_t = cst.tile([65, 128], fr, tag="sel")
            xn1 = cst.tile([64, CHUNK], fr, tag="xn1")
            xn_full = cst.tile([128, CHUNK], fr, tag="xnf")
            bp_sb = cst.tile([128, 2, CHUNK], f32, tag="bpsb")
            o_sb = cst.tile([128, CHUNK], f32, tag="osb")
            kout_sb = cst.tile([128, CHUNK], f32, tag="kout")
            vout_sb = cst.tile([128, 4, 128], f32, tag="vout")

            # ---- input DMAs: few big transfers; cache streams on the
            # gpsimd queue so it doesn't serialize behind the x/w loads ----
            xq_r = xq_d[:].rearrange("k p n -> p k n")
            xk_r = xk_d[:].rearrange("k p n -> p k n")
            xv_r = xv_d[:].rearrange("k p n -> p k n")
            KCHUNK = 1280
            vt_r = vt_d[:].rearrange("(j p) d -> p j d", p=128)

            def cache_chunk(i):
                nc.sync.dma_start(
                    out=k_sb[:, i * KCHUNK:(i + 1) * KCHUNK],
                    in_=kc_d[:, i * KCHUNK:(i + 1) * KCHUNK],
                )
                nc.sync.dma_start(
                    out=vt_sb[:, i * 10:(i + 1) * 10, :],
                    in_=vt_r[:, i * 10:(i + 1) * 10, :],
                )

            for i in range(4):
                nc.sync.dma_start(out=xq_sb[:, 2*i:2*i+2, :], in_=xq_r[:, 2*i:2*i+2, :])
            nc.sync.dma_start(out=wq_sb[:, :, :], in_=wq_d[:].rearrange("k p d -> p k d"))
            nc.sync.dma_start(out=cst32_sb[:, :], in_=cst32_d[:, :])
            cache_chunk(0)
            for i in range(4):
                nc.sync.dma_start(out=xk_sb[:, 2*i:2*i+2, :], in_=xk_r[:, 2*i:2*i+2, :])
            nc.sync.dma_start(out=wk_sb[:, :, :], in_=wk_d[:].rearrange("k p d -> p k d"))
            nc.sync.dma_start(out=cstb_sb[:, :], in_=cstb_d[:, :])
            nc.sync.dma_start(out=onec_sb[:, :], in_=onec_d[:, :])
            cache_chunk(1)
            for i in range(4):
                nc.sync.dma_start(out=xv_sb[:, 2*i:2*i+2, :], in_=xv_r[:, 2*i:2*i+2, :])
            nc.sync.dma_start(out=wv_sb[:, :, :], in_=wv_d[:].rearrange("k p d -> p k d"))
            nc.sync.dma_start(out=wo_sb[:, :, :], in_=wo_d[:].rearrange("k p d -> p k d"))
            for i in range(2, CACHE_T // KCHUNK):
                cache_chunk(i)
            # sel row (ones in first 64 cols) placed at partition 64 for the
            # broadcast matmul's lhsT
            nc.sync.dma_start(out=sel_t[64:65, :], in_=cstb_d[0:1, 256:384])

            # ---- attention (q/k/v projections interleaved into the
            # score-group stream so the PE never idles on their DMAs) ----
            with (
                tc.tile_pool(name="scp", bufs=2, space="PSUM") as scp,
                tc.tile_pool(name="xap", bufs=1, space="PSUM") as xap,
                tc.tile_pool(name="exp", bufs=6) as exp_pool,
            ):
                x0 = xap.tile([65, CHUNK], f32, tag="x0")
                x1 = xap.tile([65, CHUNK], f32, tag="x1")

                # q projection (uses one score slot, then releases it)
                q_ps = scp.tile([128, 3, CHUNK], f32, tag="sc")
                for kc in range(KCH):
                    nc.tensor.matmul(
                        q_ps[:, 0, :], wq_sb[:, kc, :], xq_sb[:, kc, :],
                        start=(kc == 0), stop=(kc == KCH - 1),
                    )
                # q = (raw + bq) * denom, with bqd = bq*denom pre-folded
                nc.scalar.activation(q_sb[:, :], q_ps[:, 0, :], Ident,
                                     bias=bqd_sb[:, 0:1], scale=DENOM)

                def k_proj():
                    k_ps = scp.tile([128, 3, CHUNK], f32, tag="sc")
                    for kc in range(KCH):
                        nc.tensor.matmul(
                            k_ps[:, 0, :], wk_sb[:, kc, :], xk_sb[:, kc, :],
                            start=(kc == 0), stop=(kc == KCH - 1),
                        )
                    nc.scalar.activation(kout_sb[:, :], k_ps[:, 0, :], Ident,
                                         bias=bk_sb[:, 0:1], scale=1.0)
                    nc.vector.tensor_copy(k_sb[:, CACHE_T:TIME], kout_sb[:, :])
                    nc.sync.dma_start(out=knew_d[:, :], in_=kout_sb[:, :])

                def v_proj(j):
                    # transposed: vT[t, d] = sum_f x[f, t] WvT[f, d] + bv[d]
                    v_ps = scp.tile([128, 3, CHUNK], f32, tag="sc")
                    vp = v_ps[:, 0, 0:128]
                    for kc in range(KCH):
                        nc.tensor.matmul(
                            vp, xv_sb[:, kc, j * 128:(j + 1) * 128],
                            wv_sb[:, kc, :],
                            start=(kc == 0), stop=False,
                        )
                    nc.tensor.matmul(vp, ones1[:, :], bv_sb[:, :],
                                     start=False, stop=True)
                    jj = NT_CACHE + j
                    nc.vector.tensor_copy(vt_sb[:, jj, 64:65], onec_sb[:, :])
                    nc.vector.tensor_copy(vt_sb[:, jj, 129:130], onec_sb[:, :])
                    nc.vector.tensor_copy(vt_sb[:, jj, 0:64], vp[:, 0:64])
                    nc.vector.tensor_copy(vt_sb[:, jj, 65:129], vp[:, 64:128])
                    nc.vector.tensor_copy(vout_sb[:, j, :], vp)
                # scores for 3 t-tiles = 6 psum banks = 2 pool slots of 3
                # banks; one Exp ACTIVATE drains 3 banks (1536 elems/lane),
                # amortizing the ~350-cycle ACT fixed cost over 1.5 tiles.
                # slot A holds (h0,j0),(h1,j0),(h0,j1); B holds
                # (h1,j1),(h0,j2),(h1,j2).
                ex_slots = {}

                def qk_pair(j, d0, d1):
                    # one t-tile's two heads, row-tiled concurrently on the
                    # PE; destinations may live in different psum slots
                    nc.tensor.matmul(
                        d0, k_sb[0:64, ts(j, 128)], q_sb[0:64, :],
                        start=True, stop=True)
                    nc.tensor.matmul(
                        d1, k_sb[64:128, ts(j, 128)], q_sb[64:128, :],
                        start=True, stop=True)

                def av(j):
                    exa, exb = ex_slots.pop(j)
                    g = j % 3
                    if g == 0:
                        r0, r1 = exa[:, 0, :], exa[:, 1, :]
                    elif g == 1:
                        r0, r1 = exa[:, 2, :], exb[:, 0, :]
                    else:
                        r0, r1 = exb[:, 1, :], exb[:, 2, :]
                    nc.tensor.matmul(
                        x0[:, :], vt_sb[:, j, 0:65], r0,
                        start=(j == 0), stop=(j == NT - 1))
                    nc.tensor.matmul(
                        x1[:, :], vt_sb[:, j, 65:130], r1,
                        start=(j == 0), stop=(j == NT - 1))

                LAG = 3  # in t-tiles, rounded up to whole 3-tile groups
                NG = NT // 3  # 21 full groups; tile 63 handled separately
                done = 0
                for g in range(NG):
                    j = 3 * g
                    scA = scp.tile([128, 3, CHUNK], f32, tag="sc")
                    scB = scp.tile([128, 3, CHUNK], f32, tag="sc")
                    qk_pair(j, scA[:, 0, :], scA[:, 1, :])
                    qk_pair(j + 1, scA[:, 2, :], scB[:, 0, :])
                    qk_pair(j + 2, scB[:, 1, :], scB[:, 2, :])
                    exa = exp_pool.tile([128, 3, CHUNK], fr, tag="ex")
                    nc.scalar.activation(exa[:, :, :], scA[:, :, :], Exp)
                    exb = exp_pool.tile([128, 3, CHUNK], fr, tag="ex")
                    nc.scalar.activation(exb[:, :, :], scB[:, :, :], Exp)
                    ex_slots[j] = (exa, exb)
                    ex_slots[j + 1] = (exa, exb)
                    ex_slots[j + 2] = (exa, exb)
                    while done <= 3 * g + 2 - LAG:
                        av(done)
                        done += 1
                    if g == 1:
                        k_proj()
                    elif 3 <= g <= 6:
                        v_proj(g - 3)
                    elif g == 7:
                        nc.sync.dma_start(
                            out=vnt_d[:].rearrange("j p d -> p j d"),
                            in_=vout_sb[:, :, :],
                        )
                # last tile (63): 2 banks in a smaller slot
                scL = scp.tile([128, 3, CHUNK], f32, tag="sc")
                qk_pair(NT - 1, scL[:, 0, :], scL[:, 1, :])
                exl = exp_pool.tile([128, 3, CHUNK], fr, tag="ex")
                nc.scalar.activation(exl[:, 0:2, :], scL[:, 0:2, :], Exp)
                ex_slots[NT - 1] = (exl, exl)
                while done < NT - 1:
                    av(done)
                    done += 1
                # final tile uses exl directly
                nc.tensor.matmul(x0[:, :], vt_sb[:, NT - 1, 0:65], exl[:, 0, :],
                                 start=False, stop=True)
                nc.tensor.matmul(x1[:, :], vt_sb[:, NT - 1, 65:130], exl[:, 1, :],
                                 start=False, stop=True)

                # ---- softmax normalization (deferred) ----
                # The sumexp rows live on one partition; a 1-lane DVE
                # reciprocal costs ~4us, so bounce them through a (128, 8)
                # layout via SBUF->SBUF DMA to use all 128 lanes.
                sr_sb = cst.tile([65, 2 * CHUNK], f32, tag="srsb")
                nc.vector.tensor_copy(sr_sb[64:65, 0:CHUNK], x0[64:65, :])
                nc.vector.tensor_copy(sr_sb[64:65, CHUNK:2 * CHUNK], x1[64:65, :])
                rr_sb = cst.tile([128, 8], f32, tag="rrsb")
                nc.sync.dma_start(out=rr_sb[:, :], in_=sr_sb[64:65, :])
                rq_sb = cst.tile([128, 8], fr, tag="rqsb")
                with nc.allow_low_precision(reason="softmax 1/sumexp in bf16"):
                    nc.vector.reciprocal(rq_sb[:, :], rr_sb[:, :])
                rs_c = cst.tile([65, 2 * CHUNK], fr, tag="rsc")
                nc.sync.dma_start(out=rs_c[64:65, :], in_=rq_sb[:, :])
                rs_t0 = rs_c[64:65, 0:CHUNK]
                rs_t1 = rs_c[64:65, CHUNK:2 * CHUNK]
                bp = scp.tile([128, 2, CHUNK], f32, tag="sc")
                nc.tensor.matmul(bp[:, 0, :], sel_t[64:65, :], rs_t0,
                                 start=True, stop=True)
                nc.tensor.matmul(bp[:, 1, :], sel_t[64:65, :], rs_t1,
                                 start=True, stop=True)
                nc.vector.tensor_copy(bp_sb[:, :, :], bp[:, :, :])
                nc.vector.tensor_mul(xn_full[0:64, :], x0[0:64, :], bp_sb[0:64, 0, :])
                nc.vector.tensor_mul(xn1[:, :], x1[0:64, :], bp_sb[0:64, 1, :])

            if DEVICE_AG:
                # all-gather x across cores, then column-parallel linear_out
                nc.sync.dma_start(out=ccin_d[0:64, :], in_=xn_full[0:64, :])
                nc.sync.dma_start(out=ccin_d[64:128, :], in_=xn1[:, :])
                nc.gpsimd.collective_compute(
                    "AllGather",
                    mybir.AluOpType.bypass,
                    replica_groups=[list(range(NCORES))],
                    ins=[ccin_d.ap().opt()],
                    outs=[ccout_d.ap().opt()],
                )
                nc.sync.dma_start(
                    out=xf_sb[:, :, :],
                    in_=ccout_d[:].rearrange("(k p) n -> p k n", p=128))
                with tc.tile_pool(name="pop", bufs=1, space="PSUM") as pop:
                    o_ps = pop.tile([128, CHUNK], f32, tag="o_ps")
                    for kc in range(KCH):
                        nc.tensor.matmul(
                            o_ps[:, :], wo_sb[:, kc, :], xf_sb[:, kc, :],
                            start=(kc == 0), stop=(kc == KCH - 1),
                        )
                    nc.scalar.activation(o_sb[:, :], o_ps[:, :], Ident,
                                         bias=bo_sb[:, 0:1], scale=1.0)
                nc.sync.dma_start(out=out_d[:, :], in_=o_sb[:, :])
            else:
                # row-parallel linear_out: this core's 128 features produce a
                # full (1024, 512) partial; the host sums the 8 partials.
                nc.sync.dma_start(out=xn_full[64:128, :], in_=xn1[:, :])
                o8_sb = cst.tile([128, KCH, CHUNK], fr, tag="o8")
                with tc.tile_pool(name="pop", bufs=2, space="PSUM") as pop:
                    for mb in range(KCH):
                        o_ps = pop.tile([128, CHUNK], f32, tag="o_ps")
                        nc.tensor.matmul(o_ps[:, :], wo_sb[:, mb, :], xn_full[:, :],
                                         start=True, stop=True)
                        if mb % 2 == 0:
                            nc.vector.tensor_copy(o8_sb[:, mb, :], o_ps[:, :])
                        else:
                            nc.scalar.copy(o8_sb[:, mb, :], o_ps[:, :])
                for i in range(4):
                    nc.sync.dma_start(
                        out=out_d[:].rearrange("k p n -> p k n")[:, 2*i:2*i+2, :],
                        in_=o8_sb[:, 2*i:2*i+2, :])

    nc.finalize()
    return nc


def _get_nc():
    global _NC
    if _NC is None:
        _NC = _build()
    return _NC


def _mm_np_dtype():
    if USE_BF16:
        import ml_dtypes
        return ml_dtypes.bfloat16
    return np.float32


def _prep_in_maps(q, k, v, cache, Wq, bq, Wk, bk, Wv, bv, Wo, bo):
    md = _mm_np_dtype()
    xq = np.ascontiguousarray(q.reshape(NFEAT, CHUNK)).astype(md).reshape(KCH, 128, CHUNK)
    xk = np.ascontiguousarray(k.reshape(NFEAT, CHUNK)).astype(md).reshape(KCH, 128, CHUNK)
    xv = np.ascontiguousarray(v.reshape(NFEAT, CHUNK)).astype(md).reshape(KCH, 128, CHUNK)
    in_maps = []
    for c in range(NCORES):
        rows = slice(128 * c, 128 * (c + 1))
        wq_c = np.ascontiguousarray(Wq[rows, :].T).astype(md).reshape(KCH, 128, FPC)
        wk_c = np.ascontiguousarray(Wk[rows, :].T).astype(md).reshape(KCH, 128, FPC)
        wv_c = np.ascontiguousarray(Wv[rows, :].T).astype(md).reshape(KCH, 128, FPC)
        if DEVICE_AG:
            wo_c = np.ascontiguousarray(Wo[rows, :].T).astype(md).reshape(KCH, 128, FPC)
        else:
            wo_c = np.ascontiguousarray(
                Wo[:, rows].T.reshape(128, KCH, 128).transpose(1, 0, 2)).astype(md)
        kc_c = np.ascontiguousarray(
            cache[0, 2 * c:2 * c + 2, 0:DK, :]).reshape(128, CACHE_T).astype(md)
        vt_c = np.empty((CACHE_T, 130), md)
        vt_c[:, 64] = 1.0
        vt_c[:, 129] = 1.0
        vt_c[:, 0:64] = cache[0, 2 * c, DK:2 * DK, :].T.astype(md)
        vt_c[:, 65:129] = cache[0, 2 * c + 1, DK:2 * DK, :].T.astype(md)
        cst32 = np.stack([bq[rows] * DENOM, bk[rows], bo[rows]], axis=1).astype(np.float32)
        cstb = np.zeros((1, 384), md)
        cstb[0, 0:128] = bv[rows].astype(md)
        cstb[0, 128:256] = 1.0
        cstb[0, 256:320] = 1.0
        in_maps.append({
            "xq": xq, "xk": xk, "xv": xv,
            "wq": wq_c, "wk": wk_c, "wv": wv_c, "wo": wo_c,
            "cst32": cst32,
            "cstb": cstb,
            "onec_r": np.ones((128, 1), md),
            "kcache": kc_c,
            "vtcache": vt_c,
        })
    return in_maps


def _assemble(results, cache, bo=None):
    x_out = np.empty((1, NFEAT, 1, CHUNK), np.float32)
    new_cache = np.empty((1, H, 2 * DK, TIME), np.float32)
    new_cache[0, :, 0:DK, 0:CACHE_T] = cache[0, :, 0:DK, :]
    new_cache[0, :, DK:2 * DK, 0:CACHE_T] = cache[0, :, DK:2 * DK, :]
    if not DEVICE_AG:
        acc = results[0]["out_block"].astype(np.float32).reshape(NFEAT, CHUNK).copy()
        for c in range(1, NCORES):
            acc += results[c]["out_block"].reshape(NFEAT, CHUNK)
        acc += bo[:, None]
        x_out[0, :, 0, :] = acc
    for c in range(NCORES):
        res = results[c]
        if DEVICE_AG:
            x_out[0, 128 * c:128 * (c + 1), 0, :] = res["out_block"]
        knew = res["knew"].reshape(HPC, DK, CHUNK)
        new_cache[0, 2 * c:2 * c + 2, 0:DK, CACHE_T:] = knew
        vnt = res["vnt"].reshape(4 * 128, 128)  # (t_new, 2*dk)
        new_cache[0, 2 * c, DK:2 * DK, CACHE_T:] = vnt[:, 0:64].T
        new_cache[0, 2 * c + 1, DK:2 * DK, CACHE_T:] = vnt[:, 64:128].T
    return x_out, new_cache


def _fallback(q, k, v, mask, cache, Wq, bq, Wk, bk, Wv, bv, Wo, bo):
    """Pure-numpy reference path (only used if mask is not all-ones)."""
    def lin(x, W, b):
        return np.einsum("oc,bcut->bout", W, x) + b[None, :, None, None]

    qp = lin(q, Wq, bq).reshape(1, H, DK, CHUNK)
    kp = lin(k, Wk, bk).reshape(1, H, DK, CHUNK)
    vp = lin(v, Wv, bv).reshape(1, H, DK, CHUNK)
    qp = np.transpose(qp, (0, 1, 3, 2))
    k_cache, v_cache = cache[:, :, 0:DK, :], cache[:, :, DK:, :]
    kk = np.concatenate([k_cache, kp], axis=3)
    vv = np.concatenate([v_cache, vp], axis=3)
    new_cache = np.concatenate([kk, vv], axis=2)
    scores = np.einsum("bhcd,bhdt->bhct", qp, kk) * DENOM
    neg = ~mask
    scores = np.where(neg, -np.inf, scores)
    m = scores.max(axis=-1, keepdims=True)
    e = np.exp(scores - m)
    attn = e / e.sum(axis=-1, keepdims=True)
    attn = np.where(neg, 0.0, attn)
    x = np.einsum("bhdt,bhct->bhdc", vv, attn)
    x = x.reshape(1, NFEAT, 1, CHUNK)
    x_out = lin(x, Wo, bo)
    return x_out.astype(np.float32), new_cache.astype(np.float32)


def kernel(q, k, v, mask, cache, Wq, bq, Wk, bk, Wv, bv, Wo, bo):
    global LAST
    q = np.asarray(q, np.float32)
    k = np.asarray(k, np.float32)
    v = np.asarray(v, np.float32)
    cache = np.asarray(cache, np.float32)
    Wq = np.asarray(Wq, np.float32); bq = np.asarray(bq, np.float32)
    Wk = np.asarray(Wk, np.float32); bk = np.asarray(bk, np.float32)
    Wv = np.asarray(Wv, np.float32); bv = np.asarray(bv, np.float32)
    Wo = np.asarray(Wo, np.float32); bo = np.asarray(bo, np.float32)
    mask_np = np.asarray(mask)
    if not mask_np.all():
        return _fallback(q, k, v, mask_np, cache, Wq, bq, Wk, bk, Wv, bv, Wo, bo)

    from concourse.bass_utils import run_bass_kernel_spmd

    nc = _get_nc()
    in_maps = _prep_in_maps(q, k, v, cache, Wq, bq, Wk, bk, Wv, bv, Wo, bo)
    trace = os.environ.get("KERNEL_TRACE", "0") != "0"
    LAST = run_bass_kernel_spmd(nc, in_maps, list(range(NCORES)), trace=trace)
    return _assemble(LAST.results, cache, bo)


# revision 26
# speedup vs baseline: 1.0965x; 1.0079x over previous
"""Trainium2 Bass kernel for BPUMultiHeadedAttention (16 heads, dk=64,
chunk=512, time=8192) sharded over heads across 8 NeuronCores (2 heads/core).

Per-core device program (all layouts chosen so no on-device transposes are
needed):
  - q/k projections in (feat, time) orientation; v projection emitted
    transposed as (time, feat) by swapping matmul operands.
  - v-cache is transposed on the host into (time, feat) with a ones column
    appended per head, so the attention*V matmul (M=65) accumulates both
    x = v @ exp(scores) and sumexp = 1 @ exp(scores) in one PSUM tile.
  - scores are computed per 128-wide time tile in (t, c) orientation with
    the two heads row-tiled on the PE array (K=64 each), drained through
    one ScalarE Exp activation spanning both PSUM banks.
  - softmax normalization is deferred: x is divided by sumexp after the
    full accumulation (a K=1 broadcast matmul + one elementwise multiply).
  - linear_out is row-parallel: each core multiplies its 128 features
    into a full (1024, 512) partial and the host sums the 8 partials (an
    on-device AllGather variant is kept behind KERNEL_DEVICE_AG=1; the
    collective measures ~40us on this fabric, far above its documented
    cost, so the host reduction is the default).

Measured on the 8 axon-tunneled TRN2 cores: ~109 us HW exec time,
max rel err ~2e-3 (bf16 matmul operands, fp32 accumulation).
"""

import os
import math
import numpy as np

H, DK, CHUNK, LEFT = 16, 64, 512, 15
NFEAT = H * DK            # 1024
CACHE_T = CHUNK * LEFT    # 7680
TIME = CHUNK * (LEFT + 1) # 8192
DENOM = 1.0 / math.sqrt(DK)

NCORES = 8
HPC = H // NCORES         # heads per core = 2
FPC = HPC * DK            # features per core = 128
NT = TIME // 128          # 64 time tiles
NT_CACHE = CACHE_T // 128 # 60 cached time tiles
KCH = NFEAT // 128        # 8 contraction chunks

# Matmul operand dtype: bf16 (1 cyc/row on the PE + fast weight load) by
# default; f32r ("KERNEL_BF16=0") runs as two-pass fp32-HIGH at half rate.
USE_BF16 = os.environ.get("KERNEL_BF16", "1") != "0"
USE_F32R = os.environ.get("KERNEL_F32R", "1") != "0"
# Device AllGather + column-parallel linear_out vs row-parallel linear_out
# with the 8 partial sums reduced on the host (the AllGather measures ~40us
# on this fabric, an order above its documented cost, so host-reduce wins).
DEVICE_AG = os.environ.get("KERNEL_DEVICE_AG", "0") != "0"

_NC = None
LAST = None  # BassKernelResults of the most recent device run (for test.py)


def _build():
    import concourse.bass as bass
    import concourse.bacc as bacc
    import concourse.mybir as mybir
    import concourse.tile as tile
    from concourse.bass import ts

    f32 = mybir.dt.float32
    fr = mybir.dt.bfloat16 if USE_BF16 else (mybir.dt.float32r if USE_F32R else f32)

    nc = bacc.Bacc("TRN2", target_bir_lowering=False, num_devices=NCORES)

    # ---- DRAM I/O (per-core shapes) ----
    xq_d = nc.dram_tensor("xq", [KCH, 128, CHUNK], fr, kind="ExternalInput")
    xk_d = nc.dram_tensor("xk", [KCH, 128, CHUNK], fr, kind="ExternalInput")
    xv_d = nc.dram_tensor("xv", [KCH, 128, CHUNK], fr, kind="ExternalInput")
    wq_d = nc.dram_tensor("wq", [KCH, 128, FPC], fr, kind="ExternalInput")
    wk_d = nc.dram_tensor("wk", [KCH, 128, FPC], fr, kind="ExternalInput")
    wv_d = nc.dram_tensor("wv", [KCH, 128, FPC], fr, kind="ExternalInput")
    wo_d = nc.dram_tensor("wo", [KCH, 128, FPC], fr, kind="ExternalInput")
    cst32_d = nc.dram_tensor("cst32", [128, 3], f32, kind="ExternalInput")
    cstb_d = nc.dram_tensor("cstb", [1, 384], fr, kind="ExternalInput")
    onec_d = nc.dram_tensor("onec_r", [128, 1], fr, kind="ExternalInput")
    kc_d = nc.dram_tensor("kcache", [128, CACHE_T], fr, kind="ExternalInput")
    vt_d = nc.dram_tensor("vtcache", [CACHE_T, 130], fr, kind="ExternalInput")

    if DEVICE_AG:
        out_d = nc.dram_tensor("out_block", [128, CHUNK], f32, kind="ExternalOutput")
        # collective bounce buffers
        ccin_d = nc.dram_tensor("ccin", [128, CHUNK], fr)
        ccout_d = nc.dram_tensor("ccout", [NFEAT, CHUNK], fr, addr_space="Shared")
    else:
        out_d = nc.dram_tensor("out_block", [KCH, 128, CHUNK], fr, kind="ExternalOutput")
    knew_d = nc.dram_tensor("knew", [128, CHUNK], f32, kind="ExternalOutput")
    vnt_d = nc.dram_tensor("vnt", [4, 128, 128], f32, kind="ExternalOutput")

    Exp = mybir.ActivationFunctionType.Exp
    Ident = mybir.ActivationFunctionType.Identity


    with tile.TileContext(nc) as tc:
        with (
            tc.tile_pool(name="cst", bufs=1) as cst,
            tc.tile_pool(name="big", bufs=1) as big,
        ):
            # ---- persistent SBUF tiles ----
            xq_sb = big.tile([128, KCH, CHUNK], fr, tag="xq")
            xk_sb = big.tile([128, KCH, CHUNK], fr, tag="xk")
            xv_sb = big.tile([128, KCH, CHUNK], fr, tag="xv")
            wq_sb = big.tile([128, KCH, FPC], fr, tag="wq")
            wk_sb = big.tile([128, KCH, FPC], fr, tag="wk")
            wv_sb = big.tile([128, KCH, FPC], fr, tag="wv")
            wo_sb = big.tile([128, KCH, FPC], fr, tag="wo")
            k_sb = big.tile([128, TIME], fr, tag="ksb")
            vt_sb = big.tile([128, NT, 130], fr, tag="vtsb")
            q_sb = big.tile([128, CHUNK], fr, tag="qsb")
            if DEVICE_AG:
                xf_sb = big.tile([128, KCH, CHUNK], fr, tag="xf")

            cst32_sb = cst.tile([128, 3], f32, tag="cst32")
            cstb_sb = cst.tile([1, 384], fr, tag="cstb")
            onec_sb = cst.tile([128, 1], fr, tag="onec")
            bqd_sb = cst32_sb[:, 0:1]
            bk_sb = cst32_sb[:, 1:2]
            bo_sb = cst32_sb[:, 2:3]
            bv_sb = cstb_sb[0:1, 0:128]
            ones1 = cstb_sb[0:1, 128:256]
            sel_t = cst.tile([65, 128], fr, tag="sel")
            xn1 = cst.tile([64, CHUNK], fr, tag="xn1")
            xn_full = cst.tile([128, CHUNK], fr, tag="xnf")
            bp_sb = cst.tile([128, 2, CHUNK], f32, tag="bpsb")
            o_sb = cst.tile([128, CHUNK], f32, tag="osb")
            kout_sb = cst.tile([128, CHUNK], f32, tag="kout")
            vout_sb = cst.tile([128, 4, 128], f32, tag="vout")

            # ---- input DMAs: few big transfers; cache streams on the
            # gpsimd queue so it doesn't serialize behind the x/w loads ----
            xq_r = xq_d[:].rearrange("k p n -> p k n")
            xk_r = xk_d[:].rearrange("k p n -> p k n")
            xv_r = xv_d[:].rearrange("k p n -> p k n")
            KCHUNK = 1280
            vt_r = vt_d[:].rearrange("(j p) d -> p j d", p=128)

            def cache_chunk(i):
                nc.sync.dma_start(
                    out=k_sb[:, i * KCHUNK:(i + 1) * KCHUNK],
                    in_=kc_d[:, i * KCHUNK:(i + 1) * KCHUNK],
                )
                nc.sync.dma_start(
                    out=vt_sb[:, i * 10:(i + 1) * 10, :],
                    in_=vt_r[:, i * 10:(i + 1) * 10, :],
                )

            nc.sync.dma_start(out=wq_sb[:, :, :], in_=wq_d[:].rearrange("k p d -> p k d"))
            for i in range(4):
                nc.sync.dma_start(out=xq_sb[:, 2*i:2*i+2, :], in_=xq_r[:, 2*i:2*i+2, :])
            nc.sync.dma_start(out=cst32_sb[:, :], in_=cst32_d[:, :])
            cache_chunk(0)
            for i in range(4):
                nc.sync.dma_start(out=xk_sb[:, 2*i:2*i+2, :], in_=xk_r[:, 2*i:2*i+2, :])
            nc.sync.dma_start(out=wk_sb[:, :, :], in_=wk_d[:].rearrange("k p d -> p k d"))
            nc.sync.dma_start(out=cstb_sb[:, :], in_=cstb_d[:, :])
            nc.sync.dma_start(out=onec_sb[:, :], in_=onec_d[:, :])
            cache_chunk(1)
            for i in range(4):
                nc.sync.dma_start(out=xv_sb[:, 2*i:2*i+2, :], in_=xv_r[:, 2*i:2*i+2, :])
            nc.sync.dma_start(out=wv_sb[:, :, :], in_=wv_d[:].rearrange("k p d -> p k d"))
            nc.sync.dma_start(out=wo_sb[:, :, :], in_=wo_d[:].rearrange("k p d -> p k d"))
            for i in range(2, CACHE_T // KCHUNK):
                cache_chunk(i)
            # sel row (ones in first 64 cols) placed at partition 64 for the
            # broadcast matmul's lhsT
            nc.sync.dma_start(out=sel_t[64:65, :], in_=cstb_d[0:1, 256:384])

            # ---- attention (q/k/v projections interleaved into the
            # score-group stream so the PE never idles on their DMAs) ----
            with (
                tc.tile_pool(name="scp", bufs=2, space="PSUM") as scp,
                tc.tile_pool(name="xap", bufs=1, space="PSUM") as xap,
                tc.tile_pool(name="exp", bufs=8) as exp_pool,
            ):
                xacc = xap.tile([65, 2, CHUNK], f32, tag="xacc")
                x0 = xacc[:, 0, :]
                x1 = xacc[:, 1, :]

                # q projection (uses one score slot, then releases it)
                q_ps = scp.tile([128, 3, CHUNK], f32, tag="sc")
                for kc in range(KCH):
                    nc.tensor.matmul(
                        q_ps[:, 0, :], wq_sb[:, kc, :], xq_sb[:, kc, :],
                        start=(kc == 0), stop=(kc == KCH - 1),
                    )
                # q = (raw + bq) * denom, with bqd = bq*denom pre-folded
                nc.scalar.activation(q_sb[:, :], q_ps[:, 0, :], Ident,
                                     bias=bqd_sb[:, 0:1], scale=DENOM)

                def k_proj():
                    k_ps = scp.tile([128, 3, CHUNK], f32, tag="sc")
                    for kc in range(KCH):
                        nc.tensor.matmul(
                            k_ps[:, 0, :], wk_sb[:, kc, :], xk_sb[:, kc, :],
                            start=(kc == 0), stop=(kc == KCH - 1),
                        )
                    nc.scalar.activation(kout_sb[:, :], k_ps[:, 0, :], Ident,
                                         bias=bk_sb[:, 0:1], scale=1.0)
                    nc.vector.tensor_copy(k_sb[:, CACHE_T:TIME], kout_sb[:, :])
                    nc.sync.dma_start(out=knew_d[:, :], in_=kout_sb[:, :])

                def v_proj(j):
                    # transposed: vT[t, d] = sum_f x[f, t] WvT[f, d] + bv[d]
                    v_ps = scp.tile([128, 3, CHUNK], f32, tag="sc")
                    vp = v_ps[:, 0, 0:128]
                    for kc in range(KCH):
                        nc.tensor.matmul(
                            vp, xv_sb[:, kc, j * 128:(j + 1) * 128],
                            wv_sb[:, kc, :],
                            start=(kc == 0), stop=False,
                        )
                    nc.tensor.matmul(vp, ones1[:, :], bv_sb[:, :],
                                     start=False, stop=True)
                    jj = NT_CACHE + j
                    nc.vector.tensor_copy(vt_sb[:, jj, 64:65], onec_sb[:, :])
                    nc.vector.tensor_copy(vt_sb[:, jj, 129:130], onec_sb[:, :])
                    nc.vector.tensor_copy(vt_sb[:, jj, 0:64], vp[:, 0:64])
                    nc.vector.tensor_copy(vt_sb[:, jj, 65:129], vp[:, 64:128])
                    nc.vector.tensor_copy(vout_sb[:, j, :], vp)
                # scores for 3 t-tiles = 6 psum banks = 2 pool slots of 3
                # banks; one Exp ACTIVATE drains 3 banks (1536 elems/lane),
                # amortizing the ~350-cycle ACT fixed cost over 1.5 tiles.
                # slot A holds (h0,j0),(h1,j0),(h0,j1); B holds
                # (h1,j1),(h0,j2),(h1,j2).
                ex_slots = {}

                def qk_pair(j, d0, d1):
                    # one t-tile's two heads, row-tiled concurrently on the
                    # PE; destinations may live in different psum slots
                    nc.tensor.matmul(
                        d0, k_sb[0:64, ts(j, 128)], q_sb[0:64, :],
                        start=True, stop=True)
                    nc.tensor.matmul(
                        d1, k_sb[64:128, ts(j, 128)], q_sb[64:128, :],
                        start=True, stop=True)

                def av(j):
                    exa, exb = ex_slots.pop(j)
                    g = j % 3
                    if g == 0:
                        r0, r1 = exa[:, 0, :], exa[:, 1, :]
                    elif g == 1:
                        r0, r1 = exa[:, 2, :], exb[:, 0, :]
                    else:
                        r0, r1 = exb[:, 1, :], exb[:, 2, :]
                    nc.tensor.matmul(
                        x0, vt_sb[:, j, 0:65], r0,
                        start=(j == 0), stop=(j == NT - 1))
                    nc.tensor.matmul(
                        x1, vt_sb[:, j, 65:130], r1,
                        start=(j == 0), stop=(j == NT - 1))

                LAG = 3  # in t-tiles, rounded up to whole 3-tile groups
                NG = NT // 3  # 21 full groups; tile 63 handled separately
                done = 0
                for g in range(NG):
                    j = 3 * g
                    scA = scp.tile([128, 3, CHUNK], f32, tag="sc")
                    scB = scp.tile([128, 3, CHUNK], f32, tag="sc")
                    qk_pair(j, scA[:, 0, :], scA[:, 1, :])
                    qk_pair(j + 1, scA[:, 2, :], scB[:, 0, :])
                    qk_pair(j + 2, scB[:, 1, :], scB[:, 2, :])
                    exa = exp_pool.tile([128, 3, CHUNK], fr, tag="ex")
                    nc.scalar.activation(exa[:, :, :], scA[:, :, :], Exp)
                    exb = exp_pool.tile([128, 3, CHUNK], fr, tag="ex")
                    nc.scalar.activation(exb[:, :, :], scB[:, :, :], Exp)
                    ex_slots[j] = (exa, exb)
                    ex_slots[j + 1] = (exa, exb)
                    ex_slots[j + 2] = (exa, exb)
                    while done <= 3 * g + 2 - LAG:
                        av(done)
                        done += 1
                    if g == 1:
                        k_proj()
                    elif 3 <= g <= 6:
                        v_proj(g - 3)
                    elif g == 7:
                        nc.sync.dma_start(
                            out=vnt_d[:].rearrange("j p d -> p j d"),
                            in_=vout_sb[:, :, :],
                        )
                # last tile (63): 2 banks in a smaller slot
                scL = scp.tile([128, 3, CHUNK], f32, tag="sc")
                qk_pair(NT - 1, scL[:, 0, :], scL[:, 1, :])
                exl = exp_pool.tile([128, 3, CHUNK], fr, tag="ex")
                nc.scalar.activation(exl[:, 0:2, :], scL[:, 0:2, :], Exp)
                ex_slots[NT - 1] = (exl, exl)
                while done < NT - 1:
                    av(done)
                    done += 1
                # final tile uses exl directly
                nc.tensor.matmul(x0, vt_sb[:, NT - 1, 0:65], exl[:, 0, :],
                                 start=False, stop=True)
                nc.tensor.matmul(x1, vt_sb[:, NT - 1, 65:130], exl[:, 1, :],
                                 start=False, stop=True)

                # ---- softmax normalization (deferred) ----
                # The sumexp rows live on one partition; a 1-lane DVE
                # reciprocal costs ~4us, so bounce them through a (128, 8)
                # layout via SBUF->SBUF DMA to use all 128 lanes.
                sr_sb = cst.tile([65, 2 * CHUNK], f32, tag="srsb")
                nc.vector.tensor_copy(
                    sr_sb[64:65, :].rearrange("p (h n) -> p h n", h=2),
                    xacc[64:65, :, :])
                rr_sb = cst.tile([128, 8], f32, tag="rrsb")
                nc.sync.dma_start(out=rr_sb[:, :], in_=sr_sb[64:65, :])
                rq_sb = cst.tile([128, 8], fr, tag="rqsb")
                with nc.allow_low_precision(reason="softmax 1/sumexp in bf16"):
                    nc.vector.reciprocal(rq_sb[:, :], rr_sb[:, :])
                rs_c = cst.tile([65, 2 * CHUNK], fr, tag="rsc")
                nc.sync.dma_start(out=rs_c[64:65, :], in_=rq_sb[:, :])
                rs_t0 = rs_c[64:65, 0:CHUNK]
                rs_t1 = rs_c[64:65, CHUNK:2 * CHUNK]
                bp = scp.tile([128, 2, CHUNK], f32, tag="sc")
                nc.tensor.matmul(bp[:, 0, :], sel_t[64:65, :], rs_t0,
                                 start=True, stop=True)
                nc.tensor.matmul(bp[:, 1, :], sel_t[64:65, :], rs_t1,
                                 start=True, stop=True)
                nc.scalar.copy(bp_sb[:, :, :], bp[:, :, :])
                nc.vector.tensor_mul(xn_full[0:64, :], xacc[0:64, 0, :], bp_sb[0:64, 0, :])
                nc.vector.tensor_mul(xn1[:, :], xacc[0:64, 1, :], bp_sb[0:64, 1, :])

            if DEVICE_AG:
                # all-gather x across cores, then column-parallel linear_out
                nc.sync.dma_start(out=ccin_d[0:64, :], in_=xn_full[0:64, :])
                nc.sync.dma_start(out=ccin_d[64:128, :], in_=xn1[:, :])
                nc.gpsimd.collective_compute(
                    "AllGather",
                    mybir.AluOpType.bypass,
                    replica_groups=[list(range(NCORES))],
                    ins=[ccin_d.ap().opt()],
                    outs=[ccout_d.ap().opt()],
                )
                nc.sync.dma_start(
                    out=xf_sb[:, :, :],
                    in_=ccout_d[:].rearrange("(k p) n -> p k n", p=128))
                with tc.tile_pool(name="pop", bufs=1, space="PSUM") as pop:
                    o_ps = pop.tile([128, CHUNK], f32, tag="o_ps")
                    for kc in range(KCH):
                        nc.tensor.matmul(
                            o_ps[:, :], wo_sb[:, kc, :], xf_sb[:, kc, :],
                            start=(kc == 0), stop=(kc == KCH - 1),
                        )
                    nc.scalar.activation(o_sb[:, :], o_ps[:, :], Ident,
                                         bias=bo_sb[:, 0:1], scale=1.0)
                nc.sync.dma_start(out=out_d[:, :], in_=o_sb[:, :])
            else:
                # row-parallel linear_out: this core's 128 features produce a
                # full (1024, 512) partial; the host sums the 8 partials.
                nc.sync.dma_start(out=xn_full[64:128, :], in_=xn1[:, :])
                o8_sb = cst.tile([128, KCH, CHUNK], fr, tag="o8")
                with tc.tile_pool(name="pop", bufs=2, space="PSUM") as pop:
                    for mb in range(KCH):
                        o_ps = pop.tile([128, CHUNK], f32, tag="o_ps")
                        nc.tensor.matmul(o_ps[:, :], wo_sb[:, mb, :], xn_full[:, :],
                                         start=True, stop=True)
                        if mb % 2 == 0:
                            nc.vector.tensor_copy(o8_sb[:, mb, :], o_ps[:, :])
                        else:
                            nc.scalar.copy(o8_sb[:, mb, :], o_ps[:, :])
                for i in range(4):
                    nc.sync.dma_start(
                        out=out_d[:].rearrange("k p n -> p k n")[:, 2*i:2*i+2, :],
                        in_=o8_sb[:, 2*i:2*i+2, :])

    nc.finalize()
    return nc


def _get_nc():
    global _NC
    if _NC is None:
        _NC = _build()
    return _NC


def _mm_np_dtype():
    if USE_BF16:
        import ml_dtypes
        return ml_dtypes.bfloat16
    return np.float32


def _prep_in_maps(q, k, v, cache, Wq, bq, Wk, bk, Wv, bv, Wo, bo):
    md = _mm_np_dtype()
    xq = np.ascontiguousarray(q.reshape(NFEAT, CHUNK)).astype(md).reshape(KCH, 128, CHUNK)
    xk = np.ascontiguousarray(k.reshape(NFEAT, CHUNK)).astype(md).reshape(KCH, 128, CHUNK)
    xv = np.ascontiguousarray(v.reshape(NFEAT, CHUNK)).astype(md).reshape(KCH, 128, CHUNK)
    in_maps = []
    for c in range(NCORES):
        rows = slice(128 * c, 128 * (c + 1))
        wq_c = np.ascontiguousarray(Wq[rows, :].T).astype(md).reshape(KCH, 128, FPC)
        wk_c = np.ascontiguousarray(Wk[rows, :].T).astype(md).reshape(KCH, 128, FPC)
        wv_c = np.ascontiguousarray(Wv[rows, :].T).astype(md).reshape(KCH, 128, FPC)
        if DEVICE_AG:
            wo_c = np.ascontiguousarray(Wo[rows, :].T).astype(md).reshape(KCH, 128, FPC)
        else:
            wo_c = np.ascontiguousarray(
                Wo[:, rows].T.reshape(128, KCH, 128).transpose(1, 0, 2)).astype(md)
        kc_c = np.ascontiguousarray(
            cache[0, 2 * c:2 * c + 2, 0:DK, :]).reshape(128, CACHE_T).astype(md)
        vt_c = np.empty((CACHE_T, 130), md)
        vt_c[:, 64] = 1.0
        vt_c[:, 129] = 1.0
        vt_c[:, 0:64] = cache[0, 2 * c, DK:2 * DK, :].T.astype(md)
        vt_c[:, 65:129] = cache[0, 2 * c + 1, DK:2 * DK, :].T.astype(md)
        cst32 = np.stack([bq[rows] * DENOM, bk[rows], bo[rows]], axis=1).astype(np.float32)
        cstb = np.zeros((1, 384), md)
        cstb[0, 0:128] = bv[rows].astype(md)
        cstb[0, 128:256] = 1.0
        cstb[0, 256:320] = 1.0
        in_maps.append({
            "xq": xq, "xk": xk, "xv": xv,
            "wq": wq_c, "wk": wk_c, "wv": wv_c, "wo": wo_c,
            "cst32": cst32,
            "cstb": cstb,
            "onec_r": np.ones((128, 1), md),
            "kcache": kc_c,
            "vtcache": vt_c,
        })
    return in_maps


def _assemble(results, cache, bo=None):
    x_out = np.empty((1, NFEAT, 1, CHUNK), np.float32)
    new_cache = np.empty((1, H, 2 * DK, TIME), np.float32)
    new_cache[0, :, 0:DK, 0:CACHE_T] = cache[0, :, 0:DK, :]
    new_cache[0, :, DK:2 * DK, 0:CACHE_T] = cache[0, :, DK:2 * DK, :]
    if not DEVICE_AG:
        acc = results[0]["out_block"].astype(np.float32).reshape(NFEAT, CHUNK).copy()
        for c in range(1, NCORES):
            acc += results[c]["out_block"].reshape(NFEAT, CHUNK)
        acc += bo[:, None]
        x_out[0, :, 0, :] = acc
    for c in range(NCORES):
        res = results[c]
        if DEVICE_AG:
            x_out[0, 128 * c:128 * (c + 1), 0, :] = res["out_block"]
        knew = res["knew"].reshape(HPC, DK, CHUNK)
        new_cache[0, 2 * c:2 * c + 2, 0:DK, CACHE_T:] = knew
        vnt = res["vnt"].reshape(4 * 128, 128)  # (t_new, 2*dk)
        new_cache[0, 2 * c, DK:2 * DK, CACHE_T:] = vnt[:, 0:64].T
        new_cache[0, 2 * c + 1, DK:2 * DK, CACHE_T:] = vnt[:, 64:128].T
    return x_out, new_cache


def _fallback(q, k, v, mask, cache, Wq, bq, Wk, bk, Wv, bv, Wo, bo):
    """Pure-numpy reference path (only used if mask is not all-ones)."""
    def lin(x, W, b):
        return np.einsum("oc,bcut->bout", W, x) + b[None, :, None, None]

    qp = lin(q, Wq, bq).reshape(1, H, DK, CHUNK)
    kp = lin(k, Wk, bk).reshape(1, H, DK, CHUNK)
    vp = lin(v, Wv, bv).reshape(1, H, DK, CHUNK)
    qp = np.transpose(qp, (0, 1, 3, 2))
    k_cache, v_cache = cache[:, :, 0:DK, :], cache[:, :, DK:, :]
    kk = np.concatenate([k_cache, kp], axis=3)
    vv = np.concatenate([v_cache, vp], axis=3)
    new_cache = np.concatenate([kk, vv], axis=2)
    scores = np.einsum("bhcd,bhdt->bhct", qp, kk) * DENOM
    neg = ~mask
    scores = np.where(neg, -np.inf, scores)
    m = scores.max(axis=-1, keepdims=True)
    e = np.exp(scores - m)
    attn = e / e.sum(axis=-1, keepdims=True)
    attn = np.where(neg, 0.0, attn)
    x = np.einsum("bhdt,bhct->bhdc", vv, attn)
    x = x.reshape(1, NFEAT, 1, CHUNK)
    x_out = lin(x, Wo, bo)
    return x_out.astype(np.float32), new_cache.astype(np.float32)


def kernel(q, k, v, mask, cache, Wq, bq, Wk, bk, Wv, bv, Wo, bo):
    global LAST
    q = np.asarray(q, np.float32)
    k = np.asarray(k, np.float32)
    v = np.asarray(v, np.float32)
    cache = np.asarray(cache, np.float32)
    Wq = np.asarray(Wq, np.float32); bq = np.asarray(bq, np.float32)
    Wk = np.asarray(Wk, np.float32); bk = np.asarray(bk, np.float32)
    Wv = np.asarray(Wv, np.float32); bv = np.asarray(bv, np.float32)
    Wo = np.asarray(Wo, np.float32); bo = np.asarray(bo, np.float32)
    mask_np = np.asarray(mask)
    if not mask_np.all():
        return _fallback(q, k, v, mask_np, cache, Wq, bq, Wk, bk, Wv, bv, Wo, bo)

    from concourse.bass_utils import run_bass_kernel_spmd

    nc = _get_nc()
    in_maps = _prep_in_maps(q, k, v, cache, Wq, bq, Wk, bk, Wv, bv, Wo, bo)
    trace = os.environ.get("KERNEL_TRACE", "0") != "0"
    LAST = run_bass_kernel_spmd(nc, in_maps, list(range(NCORES)), trace=trace)
    return _assemble(LAST.results, cache, bo)
